# revision 1
# baseline (speedup 1.0000x reference)
"""Multi-head self-attention (B=4, S=2048, D=1024, H=16) on 8 trn2 NeuronCores.

Sharding: core c -> batch b = c//2, head-group g = c%2 (8 heads, 512 of the
1024 output/QKV columns). Each core computes Q/K/V projections for its slice
and full attention for its 8 heads. Host does layout prep (x transpose, W
column slices) and the final gather/transpose - no collectives needed.

Per-core device pipeline (all matmuls in float32r: full PE rate at N=512,
~1.6e-4 relative error):
  phase 1: QT[512,2048], KT[512,2048] = (W.T @ xT-chunks); V[2048,512] packed
           into Vx[128,16,8,65] with a ones column per head (PV denominator).
  phase 2: per head-pair, per q-chunk(512), per k-block(16):
           scoresT[k,q] psum <- KT-tile.T @ QT-chunk (2 heads -> [128,1024]);
           expT = ACT Exp(scores * 1/16) -> f32r sbuf;
           pv[65,512] psum += Vx-tile.T @ expT  (row 64 = softmax denominator)
           then normalize: out = pv[0:64] * partition_broadcast(1/pv[64]).
  output: outT[512,2048] per core; host writes out[b,:,cols] = outT.T.
"""
import numpy as np

import concourse.bacc as bacc
import concourse.mybir as mybir
import concourse.tile as tile
from concourse.bass_utils import run_bass_kernel_spmd

B, S, D, H = 4, 2048, 1024, 16
DH = D // H            # 64
NCORES = 8
HLOC = H // 2          # 8 heads per core
DLOC = HLOC * DH       # 512 output cols per core
F32 = mybir.dt.float32
F32R = mybir.dt.float32r
EXPF = mybir.ActivationFunctionType.Exp

SC = 512               # s-chunk in phase 1
QC = 512               # q-chunk in phase 2
NKB = S // 128         # 16 k-blocks
NDT = D // 128         # 8 contraction tiles for QKV


def _build():
    nc = bacc.Bacc("TRN2", target_bir_lowering=False, debug=False, num_devices=NCORES)
    xT = nc.dram_tensor("xT", [D, S], F32R, kind="ExternalInput").ap()
    Wq = nc.dram_tensor("Wq", [D, DLOC], F32R, kind="ExternalInput").ap()
    Wk = nc.dram_tensor("Wk", [D, DLOC], F32R, kind="ExternalInput").ap()
    Wv = nc.dram_tensor("Wv", [D, DLOC], F32R, kind="ExternalInput").ap()
    out = nc.dram_tensor("outT", [DLOC, S], F32, kind="ExternalOutput").ap()

    xT_t = xT.rearrange("(o p) s -> p o s", p=128)        # [128, 8, 2048]
    out_t = out.rearrange("(o p) s -> p o s", p=128)      # [128, 4, 2048]

    with tile.TileContext(nc) as tc:
        with tc.tile_pool(name="persist", bufs=1) as keep:
            qt = keep.tile([128, DLOC // 128, S], F32R)   # QT  [p, 4, 2048]
            kt = keep.tile([128, DLOC // 128, S], F32R)   # KT  [p, 4, 2048]
            vx = keep.tile([128, NKB, HLOC, DH + 1], F32R)

            # ---------------- phase 1: QKV projections -------------------
            with nc.named_scope("qkv"), \
                 tc.tile_pool(name="p1w", bufs=1) as p1w, \
                 tc.tile_pool(name="p1x", bufs=2) as p1x, \
                 tc.tile_pool(name="p1ps", bufs=3, space="PSUM") as p1ps:
                wq_sb = p1w.tile([128, NDT, DLOC], F32R)
                wk_sb = p1w.tile([128, NDT, DLOC], F32R)
                wv_sb = p1w.tile([128, NDT, DLOC], F32R)
                nc.gpsimd.dma_start(wq_sb[:], Wq.rearrange("(o p) m -> p o m", p=128))
                nc.gpsimd.dma_start(wk_sb[:], Wk.rearrange("(o p) m -> p o m", p=128))
                nc.gpsimd.dma_start(wv_sb[:], Wv.rearrange("(o p) m -> p o m", p=128))
                ones_t = p1w.tile([128, NKB, HLOC], F32)
                nc.vector.memset(ones_t[:], 1.0)
                nc.vector.tensor_copy(vx[:, :, :, DH], ones_t[:])

                xcs = []
                for sc in range(S // SC):
                    xc = p1x.tile([128, NDT, SC], F32R, tag="xc", name=f"xc{sc}")
                    nc.gpsimd.dma_start(xc[:], xT_t[:, :, sc * SC:(sc + 1) * SC])
                    xcs.append(xc)
                    for w_sb, dst in ((wk_sb, kt), (wq_sb, qt)):
                        for m in range(DLOC // 128):
                            ps = p1ps.tile([128, SC], F32, tag="qk")
                            for dt_i in range(NDT):
                                nc.tensor.matmul(
                                    ps[:],
                                    w_sb[:, dt_i, m * 128:(m + 1) * 128],
                                    xc[:, dt_i, :],
                                    start=(dt_i == 0), stop=(dt_i == NDT - 1),
                                )
                            nc.vector.tensor_copy(
                                dst[:, m, sc * SC:(sc + 1) * SC], ps[:])
                for sc in range(S // SC):
                    xc = p1x.tile([128, NDT, SC], F32R, tag="xc", name=f"xcv{sc}")
                    nc.gpsimd.dma_start(xc[:], xT_t[:, :, sc * SC:(sc + 1) * SC])
                    for sb in range(SC // 128):
                        ps = p1ps.tile([128, DLOC], F32, tag="v")
                        for dt_i in range(NDT):
                            nc.tensor.matmul(
                                ps[:],
                                xc[:, dt_i, sb * 128:(sb + 1) * 128],
                                wv_sb[:, dt_i, :],
                                start=(dt_i == 0), stop=(dt_i == NDT - 1),
                            )
                        s_idx = sc * (SC // 128) + sb
                        nc.vector.tensor_copy(
                            vx[:, s_idx, :, 0:DH],
                            ps[:].rearrange("p (h d) -> p h d", h=HLOC))

            # ---------------- phase 2: attention -------------------------
            with nc.named_scope("attn"), \
                 tc.tile_pool(name="p2o", bufs=1) as p2o, \
                 tc.tile_pool(name="p2e", bufs=6) as p2e, \
                 tc.tile_pool(name="p2n", bufs=2) as p2n, \
                 tc.tile_pool(name="ps_s", bufs=4, space="PSUM") as ps_s, \
                 tc.tile_pool(name="ps_pv", bufs=2, space="PSUM") as ps_pv:
                ot = p2o.tile([128, DLOC // 128, S], F32)
                for hp in range(HLOC // 2):
                    for qc in range(S // QC):
                        qs = slice(qc * QC, (qc + 1) * QC)
                        pvs = [ps_pv.tile([DH + 1, QC], F32, tag=f"pv{h}",
                                          name=f"pv{h}") for h in range(2)]
                        for kb in range(NKB):
                            ks = slice(kb * 128, (kb + 1) * 128)
                            for h in range(2):
                                sp = ps_s.tile([128, QC], F32, tag="sc",
                                               name=f"sp{h}")
                                nc.tensor.matmul(
                                    sp[:],
                                    kt[64 * h:64 * h + 64, hp, ks],
                                    qt[64 * h:64 * h + 64, hp, qs],
                                    start=True, stop=True,
                                    tile_position=(64 * h, 0))
                                ex = p2e.tile([128, QC], F32R, tag="ex",
                                              name=f"ex{h}")
                                nc.scalar.activation(ex[:], sp[:], EXPF,
                                                     scale=1.0 / H)
                                nc.tensor.matmul(
                                    pvs[h][:], vx[:, kb, 2 * hp + h, :], ex[:],
                                    start=(kb == 0), stop=(kb == NKB - 1),
                                    skip_group_check=True)
                        for half, pv in ((0, pvs[0]), (1, pvs[1])):
                            dr = p2n.tile([1, QC], F32, tag="dr", name="dr")
                            nc.vector.tensor_copy(dr[:], pv[DH:DH + 1, :])
                            den = p2n.tile([1, QC], F32, tag="den", name="den")
                            nc.vector.reciprocal_approx_fast(den[:], dr[:])
                            bc = p2n.tile([DH, QC], F32, tag="bc", name="bc")
                            nc.gpsimd.partition_broadcast(bc[:], den[:])
                            nc.vector.tensor_mul(
                                ot[64 * half:64 * half + 64, hp, qs],
                                pv[0:DH, :], bc[:])
                    nc.gpsimd.dma_start(out_t[:, hp, :], ot[:, hp, :])

    nc.compile()
    return nc


def run(inputs, trace=False):
    x = np.asarray(inputs["encoder_input"], dtype=np.float32)
    Wq = np.asarray(inputs["Wq"], dtype=np.float32)
    Wk = np.asarray(inputs["Wk"], dtype=np.float32)
    Wv = np.asarray(inputs["Wv"], dtype=np.float32)

    nc = _build()
    in_maps = []
    for c in range(NCORES):
        b, g = c // 2, c % 2
        cols = slice(g * DLOC, (g + 1) * DLOC)
        in_maps.append({
            "xT": np.ascontiguousarray(x[b].T),
            "Wq": np.ascontiguousarray(Wq[:, cols]),
            "Wk": np.ascontiguousarray(Wk[:, cols]),
            "Wv": np.ascontiguousarray(Wv[:, cols]),
        })
    res = run_bass_kernel_spmd(nc, in_maps, core_ids=list(range(NCORES)),
                               trace=trace)
    out = np.empty((B, S, D), dtype=np.float32)
    for c in range(NCORES):
        b, g = c // 2, c % 2
        out[b, :, g * DLOC:(g + 1) * DLOC] = res.results[c]["outT"].T
    return out, res


def kernel(**inputs):
    out, _ = run(inputs, trace=False)
    return out



# revision 2
# speedup vs baseline: 2.0402x; 2.0402x over previous
"""Multi-head self-attention (B=4, S=2048, D=1024, H=16) on 8 trn2 NeuronCores.

Sharding: core c -> batch b = c//2, head-group g = c%2 (8 heads, 512 of the
1024 output/QKV columns). Each core computes Q/K/V projections for its slice
and full attention for its 8 heads. Host does layout prep (bf16 conversion,
x transpose, W column slices) and the final gather/transpose - no collectives.

v2 vs v1: all matmuls in bf16 (v1's float32r lowered to fp32_mode=HIGH
multi-pass matmuls at ~2-4x the cost; the trace showed PE 100% busy for
680us of the 830us attn phase). Single x pass (V uses x as stationary, Q/K
as moving, same resident SBUF copy). Exp batched to 1024 free-dim per
ACTIVATE (halves the per-instruction overhead on the Scalar engine).

Per-core pipeline:
  phase 1 (qkv): V[s,dloc] psum groups (stationary=x chunk, moving=Wv) ->
           vx[128,16,8,65] bf16 with a ones column per head (PV denominator);
           KT/QT[128(2 heads x 64 dh), hp, s] bf16 via (stationary=W chunk,
           moving=x chunk) psum groups.
  phase 2 (attn): per (hp, qc): 16 k-blocks:
           scoresT pair (2 heads, tile_position row split) -> sp[128,2,512]
           psum (2 banks); one ACTIVATE Exp(scale=1/16) -> ex[128,2,512]
           bf16; 2 PV matmuls accumulate pv[65,512] (row 64 = denominator);
           then normalize: out = pv[0:64] * partition_broadcast(1/pv[64]).
"""
import ml_dtypes
import numpy as np

import concourse.bacc as bacc
import concourse.mybir as mybir
import concourse.tile as tile
from concourse.bass_utils import run_bass_kernel_spmd

B, S, D, H = 4, 2048, 1024, 16
DH = D // H            # 64
NCORES = 8
HLOC = H // 2          # 8 heads per core
DLOC = HLOC * DH       # 512 output cols per core
F32 = mybir.dt.float32
BF16 = mybir.dt.bfloat16
EXPF = mybir.ActivationFunctionType.Exp

SC = 512               # s-chunk in phase 1
NSC = S // SC          # 4
NKB = S // 128         # 16 k-blocks
NDT = D // 128         # 8 contraction tiles for QKV
NHP = HLOC // 2        # 4 head pairs


def _build():
    nc = bacc.Bacc("TRN2", target_bir_lowering=False, debug=False,
                   num_devices=NCORES)
    # x: [p, sc, o, s'] with d = o*128+p, s = sc*512+s' (contiguous DMA rows)
    x_h = nc.dram_tensor("x4", [128, NSC, NDT, SC], BF16,
                         kind="ExternalInput").ap()
    wq_h = nc.dram_tensor("Wq", [128, NDT, DLOC], BF16,
                          kind="ExternalInput").ap()
    wk_h = nc.dram_tensor("Wk", [128, NDT, DLOC], BF16,
                          kind="ExternalInput").ap()
    wv_h = nc.dram_tensor("Wv", [128, NDT, DLOC], BF16,
                          kind="ExternalInput").ap()
    out = nc.dram_tensor("outT", [DLOC, S], F32, kind="ExternalOutput").ap()
    out_t = out.rearrange("(o p) s -> p o s", p=128)      # [128, 4, 2048]

    with tile.TileContext(nc) as tc:
        with tc.tile_pool(name="persist", bufs=1) as keep:
            x_sb = keep.tile([128, NSC, NDT, SC], BF16)
            wq_sb = keep.tile([128, NDT, DLOC], BF16)
            wk_sb = keep.tile([128, NDT, DLOC], BF16)
            wv_sb = keep.tile([128, NDT, DLOC], BF16)
            vx = keep.tile([128, NKB, HLOC, DH + 1], BF16)
            kt = keep.tile([128, NHP, S], BF16)   # [2x64 dh, hp, s]
            qt = keep.tile([128, NHP, S], BF16)
            ot = keep.tile([128, NHP, S], F32)

            nc.gpsimd.dma_start(wv_sb[:], wv_h)
            nc.gpsimd.dma_start(wk_sb[:], wk_h)
            nc.gpsimd.dma_start(wq_sb[:], wq_h)
            for sc in range(NSC):
                nc.gpsimd.dma_start(x_sb[:, sc], x_h[:, sc])
            ones_t = keep.tile([128, NKB, HLOC], BF16)
            nc.vector.memset(ones_t[:], 1.0)
            nc.vector.tensor_copy(vx[:, :, :, DH], ones_t[:])

            # ---------------- phase 1: QKV projections -------------------
            with nc.named_scope("qkv"), \
                 tc.tile_pool(name="p1ps", bufs=3, space="PSUM") as p1ps:
                for sc in range(NSC):
                    for sb in range(SC // 128):
                        ps = p1ps.tile([128, DLOC], F32, tag="p1", name="psv")
                        for dt_i in range(NDT):
                            nc.tensor.matmul(
                                ps[:],
                                x_sb[:, sc, dt_i, sb * 128:(sb + 1) * 128],
                                wv_sb[:, dt_i, :],
                                start=(dt_i == 0), stop=(dt_i == NDT - 1),
                            )
                        s_idx = sc * (SC // 128) + sb
                        nc.vector.tensor_copy(
                            vx[:, s_idx, :, 0:DH],
                            ps[:].rearrange("p (h d) -> p h d", h=HLOC))
                for hp in range(NHP):
                    cs = slice(hp * 128, (hp + 1) * 128)
                    for sc in range(NSC):
                        ss = slice(sc * SC, (sc + 1) * SC)
                        for w_sb, dst in ((wk_sb, kt), (wq_sb, qt)):
                            ps = p1ps.tile([128, SC], F32, tag="p1",
                                           name="psqk")
                            for dt_i in range(NDT):
                                nc.tensor.matmul(
                                    ps[:],
                                    w_sb[:, dt_i, cs],
                                    x_sb[:, sc, dt_i, :],
                                    start=(dt_i == 0), stop=(dt_i == NDT - 1),
                                )
                            nc.vector.tensor_copy(dst[:, hp, ss], ps[:])

            # ---------------- phase 2: attention -------------------------
            with nc.named_scope("attn"), \
                 tc.tile_pool(name="spp", bufs=2, space="PSUM") as spp, \
                 tc.tile_pool(name="pvp", bufs=2, space="PSUM") as pvp, \
                 tc.tile_pool(name="exp", bufs=4) as exp_pool, \
                 tc.tile_pool(name="nrm", bufs=2) as nrm:
                for hp in range(NHP):
                    for qc in range(NSC):
                        qs = slice(qc * SC, (qc + 1) * SC)
                        pvs = [pvp.tile([DH + 1, SC], F32, tag=f"pv{h}",
                                        name=f"pv{h}") for h in range(2)]
                        for kb in range(NKB):
                            ks = slice(kb * 128, (kb + 1) * 128)
                            sp = spp.tile([128, 2, SC], F32, tag="sp",
                                          name="sp")
                            for h in range(2):
                                nc.tensor.matmul(
                                    sp[:, h, :],
                                    kt[64 * h:64 * h + 64, hp, ks],
                                    qt[64 * h:64 * h + 64, hp, qs],
                                    start=True, stop=True,
                                    tile_position=(64 * h, 0))
                            ex = exp_pool.tile([128, 2, SC], BF16, tag="ex",
                                               name="ex")
                            nc.scalar.activation(ex[:], sp[:], EXPF,
                                                 scale=1.0 / H)
                            for h in range(2):
                                nc.tensor.matmul(
                                    pvs[h][:], vx[:, kb, 2 * hp + h, :],
                                    ex[:, h, :],
                                    start=(kb == 0), stop=(kb == NKB - 1),
                                    skip_group_check=True)
                        for h, pv in ((0, pvs[0]), (1, pvs[1])):
                            dr = nrm.tile([1, SC], F32, tag="dr", name="dr")
                            nc.vector.tensor_copy(dr[:], pv[DH:DH + 1, :])
                            den = nrm.tile([1, SC], F32, tag="den", name="den")
                            nc.vector.reciprocal_approx_fast(den[:], dr[:])
                            bc = nrm.tile([DH, SC], F32, tag="bc", name="bc")
                            nc.gpsimd.partition_broadcast(bc[:], den[:])
                            nc.vector.tensor_mul(
                                ot[64 * h:64 * h + 64, hp, qs],
                                pv[0:DH, :], bc[:])
                    nc.gpsimd.dma_start(out_t[:, hp, :], ot[:, hp, :])

    nc.compile()
    return nc


def run(inputs, trace=False):
    x = np.asarray(inputs["encoder_input"], dtype=np.float32)
    Wq = np.asarray(inputs["Wq"], dtype=np.float32)
    Wk = np.asarray(inputs["Wk"], dtype=np.float32)
    Wv = np.asarray(inputs["Wv"], dtype=np.float32)
    bf = ml_dtypes.bfloat16

    nc = _build()
    in_maps = []
    for c in range(NCORES):
        b, g = c // 2, c % 2
        cols = slice(g * DLOC, (g + 1) * DLOC)
        xT = x[b].T                                       # [1024, 2048]
        x4 = xT.reshape(NDT, 128, NSC, SC).transpose(1, 2, 0, 3)
        in_maps.append({
            "x4": np.ascontiguousarray(x4.astype(bf)),
            "Wq": np.ascontiguousarray(
                Wq[:, cols].reshape(NDT, 128, DLOC).transpose(1, 0, 2)
                .astype(bf)),
            "Wk": np.ascontiguousarray(
                Wk[:, cols].reshape(NDT, 128, DLOC).transpose(1, 0, 2)
                .astype(bf)),
            "Wv": np.ascontiguousarray(
                Wv[:, cols].reshape(NDT, 128, DLOC).transpose(1, 0, 2)
                .astype(bf)),
        })
    res = run_bass_kernel_spmd(nc, in_maps, core_ids=list(range(NCORES)),
                               trace=trace)
    out = np.empty((B, S, D), dtype=np.float32)
    for c in range(NCORES):
        b, g = c // 2, c % 2
        out[b, :, g * DLOC:(g + 1) * DLOC] = res.results[c]["outT"].T
    return out, res


def kernel(**inputs):
    out, _ = run(inputs, trace=False)
    return out


# revision 4
# speedup vs baseline: 2.1563x; 1.0569x over previous
"""Multi-head self-attention (B=4, S=2048, D=1024, H=16) on 8 trn2 NeuronCores.

Sharding: core c -> batch b = c//2, head-group g = c%2 (8 heads, 512 of the
1024 output/QKV columns). Each core computes Q/K/V projections for its slice
and full attention for its 8 heads. Host does layout prep (bf16 conversion,
x transpose, W column slices) and the final gather/transpose - no collectives.

v2 vs v1: all matmuls in bf16 (v1's float32r lowered to fp32_mode=HIGH
multi-pass matmuls at ~2-4x the cost; the trace showed PE 100% busy for
680us of the 830us attn phase). Single x pass (V uses x as stationary, Q/K
as moving, same resident SBUF copy). Exp batched to 1024 free-dim per
ACTIVATE (halves the per-instruction overhead on the Scalar engine).

Per-core pipeline:
  phase 1 (qkv): V[s,dloc] psum groups (stationary=x chunk, moving=Wv) ->
           vx[128,16,8,65] bf16 with a ones column per head (PV denominator);
           KT/QT[128(2 heads x 64 dh), hp, s] bf16 via (stationary=W chunk,
           moving=x chunk) psum groups.
  phase 2 (attn): per (hp, qc): 16 k-blocks:
           scoresT pair (2 heads, tile_position row split) -> sp[128,2,512]
           psum (2 banks); one ACTIVATE Exp(scale=1/16) -> ex[128,2,512]
           bf16; 2 PV matmuls accumulate pv[65,512] (row 64 = denominator);
           then normalize: out = pv[0:64] * partition_broadcast(1/pv[64]).
"""
import ml_dtypes
import numpy as np

import concourse.bacc as bacc
import concourse.mybir as mybir
import concourse.tile as tile
from concourse.bass_utils import run_bass_kernel_spmd

B, S, D, H = 4, 2048, 1024, 16
DH = D // H            # 64
NCORES = 8
HLOC = H // 2          # 8 heads per core
DLOC = HLOC * DH       # 512 output cols per core
F32 = mybir.dt.float32
BF16 = mybir.dt.bfloat16
EXPF = mybir.ActivationFunctionType.Exp

SC = 512               # s-chunk in phase 1
NSC = S // SC          # 4
NKB = S // 128         # 16 k-blocks
NDT = D // 128         # 8 contraction tiles for QKV
NHP = HLOC // 2        # 4 head pairs


def _build():
    nc = bacc.Bacc("TRN2", target_bir_lowering=False, debug=False,
                   num_devices=NCORES)
    # x: [p, sc, o, s'] with d = o*128+p, s = sc*512+s' (contiguous DMA rows)
    x_h = nc.dram_tensor("x4", [128, NSC, NDT, SC], BF16,
                         kind="ExternalInput").ap()
    wq_h = nc.dram_tensor("Wq", [128, NDT, DLOC], BF16,
                          kind="ExternalInput").ap()
    wk_h = nc.dram_tensor("Wk", [128, NDT, DLOC], BF16,
                          kind="ExternalInput").ap()
    wv_h = nc.dram_tensor("Wv", [128, NDT, DLOC], BF16,
                          kind="ExternalInput").ap()
    out = nc.dram_tensor("outT", [DLOC, S], F32, kind="ExternalOutput").ap()
    out_t = out.rearrange("(o p) s -> p o s", p=128)      # [128, 4, 2048]

    with tile.TileContext(nc) as tc:
        with tc.tile_pool(name="persist", bufs=1) as keep:
            x_sb = keep.tile([128, NSC, NDT, SC], BF16)
            wq_sb = keep.tile([128, NDT, DLOC], BF16)
            wk_sb = keep.tile([128, NDT, DLOC], BF16)
            wv_sb = keep.tile([128, NDT, DLOC], BF16)
            vx = keep.tile([128, NKB, HLOC, DH + 1], BF16)
            kt = keep.tile([128, NHP, S], BF16)   # [2x64 dh, hp, s]
            qt = keep.tile([128, NHP, S], BF16)
            ot = keep.tile([128, NHP, S], F32)

            nc.gpsimd.dma_start(wv_sb[:], wv_h)
            for sc in range(NSC):
                nc.gpsimd.dma_start(x_sb[:, sc], x_h[:, sc])
            nc.gpsimd.dma_start(wk_sb[:], wk_h)
            nc.gpsimd.dma_start(wq_sb[:], wq_h)
            ones_t = keep.tile([128, NKB, HLOC], BF16)
            nc.vector.memset(ones_t[:], 1.0)
            nc.vector.tensor_copy(vx[:, :, :, DH], ones_t[:])

            with tc.tile_pool(name="p1ps", bufs=2, space="PSUM") as p1ps, \
                 tc.tile_pool(name="spp", bufs=2, space="PSUM") as spp, \
                 tc.tile_pool(name="pvp", bufs=1, space="PSUM") as pvp, \
                 tc.tile_pool(name="exp", bufs=4) as exp_pool, \
                 tc.tile_pool(name="nrm", bufs=2) as nrm:

                def qk_group(hp, w_sb, dst, sc):
                    cs = slice(hp * 128, (hp + 1) * 128)
                    ss = slice(sc * SC, (sc + 1) * SC)
                    ps = p1ps.tile([128, SC], F32, tag="p1", name="psqk")
                    for dt_i in range(NDT):
                        nc.tensor.matmul(
                            ps[:],
                            w_sb[:, dt_i, cs],
                            x_sb[:, sc, dt_i, :],
                            start=(dt_i == 0), stop=(dt_i == NDT - 1),
                        )
                    nc.vector.tensor_copy(dst[:, hp, ss], ps[:])

                # -------- phase 1 head start: V (all) + Q/K for hp0 ------
                with nc.named_scope("qkv"):
                    for sc in range(NSC):
                        for sb in range(SC // 128):
                            ps = p1ps.tile([128, DLOC], F32, tag="p1",
                                           name="psv")
                            for dt_i in range(NDT):
                                nc.tensor.matmul(
                                    ps[:],
                                    x_sb[:, sc, dt_i,
                                         sb * 128:(sb + 1) * 128],
                                    wv_sb[:, dt_i, :],
                                    start=(dt_i == 0), stop=(dt_i == NDT - 1),
                                )
                            s_idx = sc * (SC // 128) + sb
                            nc.vector.tensor_copy(
                                vx[:, s_idx, :, 0:DH],
                                ps[:].rearrange("p (h d) -> p h d", h=HLOC))
                    for sc in range(NSC):
                        qk_group(0, wk_sb, kt, sc)
                        qk_group(0, wq_sb, qt, sc)

                # -------- phase 2: attention (QK for hp+1 interleaved) ---
                with nc.named_scope("attn"):
                    for hp in range(NHP):
                        for qc in range(NSC):
                            qs = slice(qc * SC, (qc + 1) * SC)
                            pvs = [pvp.tile([DH + 1, SC], F32, tag=f"pv{h}",
                                            name=f"pv{h}") for h in range(2)]
                            for kb in range(NKB):
                                ks = slice(kb * 128, (kb + 1) * 128)
                                sp = spp.tile([128, 2, SC], F32, tag="sp",
                                              name="sp")
                                for h in range(2):
                                    nc.tensor.matmul(
                                        sp[:, h, :],
                                        kt[64 * h:64 * h + 64, hp, ks],
                                        qt[64 * h:64 * h + 64, hp, qs],
                                        start=True, stop=True,
                                        tile_position=(64 * h, 0))
                                ex = exp_pool.tile([128, 2, SC], BF16,
                                                   tag="ex", name="ex")
                                nc.scalar.activation(ex[:], sp[:], EXPF,
                                                     scale=1.0 / H)
                                for h in range(2):
                                    nc.tensor.matmul(
                                        pvs[h][:], vx[:, kb, 2 * hp + h, :],
                                        ex[:, h, :],
                                        start=(kb == 0), stop=(kb == NKB - 1),
                                        skip_group_check=True)
                                # hide next head-pair's Q/K projections in
                                # the ACT-paced slack of the kb loop
                                if hp < NHP - 1 and kb in (5, 11):
                                    w_sb, dst = ((wk_sb, kt) if kb == 5
                                                 else (wq_sb, qt))
                                    qk_group(hp + 1, w_sb, dst, qc)
                            for h in range(2):
                                dr = nrm.tile([1, SC], F32, tag="dr",
                                              name="dr")
                                nc.vector.tensor_copy(dr[:],
                                                      pvs[h][DH:DH + 1, :])
                                den = nrm.tile([1, SC], F32, tag="den",
                                               name="den")
                                nc.vector.reciprocal_approx_fast(den[:],
                                                                 dr[:])
                                bc = nrm.tile([DH, SC], F32, tag="bc",
                                              name="bc")
                                nc.gpsimd.partition_broadcast(bc[:], den[:])
                                nc.vector.tensor_mul(
                                    ot[64 * h:64 * h + 64, hp, qs],
                                    pvs[h][0:DH, :], bc[:])
                        nc.gpsimd.dma_start(out_t[:, hp, :], ot[:, hp, :])

    nc.compile()
    return nc


def run(inputs, trace=False):
    x = np.asarray(inputs["encoder_input"], dtype=np.float32)
    Wq = np.asarray(inputs["Wq"], dtype=np.float32)
    Wk = np.asarray(inputs["Wk"], dtype=np.float32)
    Wv = np.asarray(inputs["Wv"], dtype=np.float32)
    bf = ml_dtypes.bfloat16

    nc = _build()
    in_maps = []
    for c in range(NCORES):
        b, g = c // 2, c % 2
        cols = slice(g * DLOC, (g + 1) * DLOC)
        xT = x[b].T                                       # [1024, 2048]
        x4 = xT.reshape(NDT, 128, NSC, SC).transpose(1, 2, 0, 3)
        in_maps.append({
            "x4": np.ascontiguousarray(x4.astype(bf)),
            "Wq": np.ascontiguousarray(
                Wq[:, cols].reshape(NDT, 128, DLOC).transpose(1, 0, 2)
                .astype(bf)),
            "Wk": np.ascontiguousarray(
                Wk[:, cols].reshape(NDT, 128, DLOC).transpose(1, 0, 2)
                .astype(bf)),
            "Wv": np.ascontiguousarray(
                Wv[:, cols].reshape(NDT, 128, DLOC).transpose(1, 0, 2)
                .astype(bf)),
        })
    res = run_bass_kernel_spmd(nc, in_maps, core_ids=list(range(NCORES)),
                               trace=trace)
    out = np.empty((B, S, D), dtype=np.float32)
    for c in range(NCORES):
        b, g = c // 2, c % 2
        out[b, :, g * DLOC:(g + 1) * DLOC] = res.results[c]["outT"].T
    return out, res


def kernel(**inputs):
    out, _ = run(inputs, trace=False)
    return out


# revision 5
# speedup vs baseline: 2.3093x; 1.0710x over previous
"""Multi-head self-attention (B=4, S=2048, D=1024, H=16) on 8 trn2 NeuronCores.

Sharding: core c -> batch b = c//2, head-group g = c%2 (8 heads, 512 of the
1024 output/QKV columns). Each core computes Q/K/V projections for its slice
and full attention for its 8 heads. Host does layout prep (bf16 conversion,
x transpose, W column slices) and the final gather/transpose - no collectives.

v2 vs v1: all matmuls in bf16 (v1's float32r lowered to fp32_mode=HIGH
multi-pass matmuls at ~2-4x the cost; the trace showed PE 100% busy for
680us of the 830us attn phase). Single x pass (V uses x as stationary, Q/K
as moving, same resident SBUF copy). Exp batched to 1024 free-dim per
ACTIVATE (halves the per-instruction overhead on the Scalar engine).

Per-core pipeline:
  phase 1 (qkv): V[s,dloc] psum groups (stationary=x chunk, moving=Wv) ->
           vx[128,16,8,65] bf16 with a ones column per head (PV denominator);
           KT/QT[128(2 heads x 64 dh), hp, s] bf16 via (stationary=W chunk,
           moving=x chunk) psum groups.
  phase 2 (attn): per (hp, qc): 16 k-blocks:
           scoresT pair (2 heads, tile_position row split) -> sp[128,2,512]
           psum (2 banks); one ACTIVATE Exp(scale=1/16) -> ex[128,2,512]
           bf16; 2 PV matmuls accumulate pv[65,512] (row 64 = denominator);
           then normalize: out = pv[0:64] * partition_broadcast(1/pv[64]).
"""
import ml_dtypes
import numpy as np

import concourse.bacc as bacc
import concourse.mybir as mybir
import concourse.tile as tile
from concourse.bass_utils import run_bass_kernel_spmd

B, S, D, H = 4, 2048, 1024, 16
DH = D // H            # 64
NCORES = 8
HLOC = H // 2          # 8 heads per core
DLOC = HLOC * DH       # 512 output cols per core
F32 = mybir.dt.float32
BF16 = mybir.dt.bfloat16
EXPF = mybir.ActivationFunctionType.Exp

SC = 512               # s-chunk in phase 1
NSC = S // SC          # 4
NKB = S // 128         # 16 k-blocks
NDT = D // 128         # 8 contraction tiles for QKV
NHP = HLOC // 2        # 4 head pairs


def _build():
    nc = bacc.Bacc("TRN2", target_bir_lowering=False, debug=False,
                   num_devices=NCORES)
    # x: [p, sc, o, s'] with d = o*128+p, s = sc*512+s' (contiguous DMA rows)
    x_h = nc.dram_tensor("x4", [128, NSC, NDT, SC], BF16,
                         kind="ExternalInput").ap()
    wq_h = nc.dram_tensor("Wq", [128, NDT, DLOC], BF16,
                          kind="ExternalInput").ap()
    wk_h = nc.dram_tensor("Wk", [128, NDT, DLOC], BF16,
                          kind="ExternalInput").ap()
    wv_h = nc.dram_tensor("Wv", [128, NDT, DLOC], BF16,
                          kind="ExternalInput").ap()
    out = nc.dram_tensor("outT", [DLOC, S], F32, kind="ExternalOutput").ap()
    out_t = out.rearrange("(o p) s -> p o s", p=128)      # [128, 4, 2048]

    with tile.TileContext(nc) as tc:
        with tc.tile_pool(name="persist", bufs=1) as keep:
            x_sb = keep.tile([128, NSC, NDT, SC], BF16)
            wq_sb = keep.tile([128, NDT, DLOC], BF16)
            wk_sb = keep.tile([128, NDT, DLOC], BF16)
            wv_sb = keep.tile([128, NDT, DLOC], BF16)
            vx = keep.tile([128, NKB, HLOC, DH + 1], BF16)
            kt = keep.tile([128, NHP, S], BF16)   # [2x64 dh, hp, s]
            qt = keep.tile([128, NHP, S], BF16)
            ot = keep.tile([128, NHP, S], F32)

            nc.gpsimd.dma_start(wv_sb[:], wv_h)
            for sc in range(NSC):
                nc.gpsimd.dma_start(x_sb[:, sc], x_h[:, sc])
            nc.gpsimd.dma_start(wk_sb[:], wk_h)
            nc.gpsimd.dma_start(wq_sb[:], wq_h)
            ones_t = keep.tile([128, NKB, HLOC], BF16)
            nc.vector.memset(ones_t[:], 1.0)
            nc.vector.tensor_copy(vx[:, :, :, DH], ones_t[:])

            with tc.tile_pool(name="p1ps", bufs=2, space="PSUM") as p1ps, \
                 tc.tile_pool(name="spp", bufs=2, space="PSUM") as spp, \
                 tc.tile_pool(name="pvp", bufs=1, space="PSUM") as pvp, \
                 tc.tile_pool(name="exp", bufs=4) as exp_pool, \
                 tc.tile_pool(name="nrm", bufs=2) as nrm:

                def qk_group(hp, w_sb, dst, sc):
                    cs = slice(hp * 128, (hp + 1) * 128)
                    ss = slice(sc * SC, (sc + 1) * SC)
                    ps = p1ps.tile([128, SC], F32, tag="p1", name="psqk")
                    for dt_i in range(NDT):
                        nc.tensor.matmul(
                            ps[:],
                            w_sb[:, dt_i, cs],
                            x_sb[:, sc, dt_i, :],
                            start=(dt_i == 0), stop=(dt_i == NDT - 1),
                        )
                    nc.vector.tensor_copy(dst[:, hp, ss], ps[:])

                # -------- phase 1 head start: V (all) + Q/K for hp0 ------
                with nc.named_scope("qkv"):
                    for sc in range(NSC):
                        for sb in range(SC // 128):
                            ps = p1ps.tile([128, DLOC], F32, tag="p1",
                                           name="psv")
                            for dt_i in range(NDT):
                                nc.tensor.matmul(
                                    ps[:],
                                    x_sb[:, sc, dt_i,
                                         sb * 128:(sb + 1) * 128],
                                    wv_sb[:, dt_i, :],
                                    start=(dt_i == 0), stop=(dt_i == NDT - 1),
                                )
                            s_idx = sc * (SC // 128) + sb
                            nc.vector.tensor_copy(
                                vx[:, s_idx, :, 0:DH],
                                ps[:].rearrange("p (h d) -> p h d", h=HLOC))
                    for sc in range(NSC):
                        qk_group(0, wk_sb, kt, sc)
                        qk_group(0, wq_sb, qt, sc)

                # -------- phase 2: attention (QK for hp+1 interleaved) ---
                with nc.named_scope("attn"):
                    for hp in range(NHP):
                        for qc in range(NSC):
                            qs = slice(qc * SC, (qc + 1) * SC)
                            pvs = [pvp.tile([DH + 1, SC], F32, tag=f"pv{h}",
                                            name=f"pv{h}") for h in range(2)]
                            for kb in range(NKB):
                                ks = slice(kb * 128, (kb + 1) * 128)
                                sp = spp.tile([128, 2, SC], F32, tag="sp",
                                              name="sp")
                                for h in range(2):
                                    nc.tensor.matmul(
                                        sp[:, h, :],
                                        kt[64 * h:64 * h + 64, hp, ks],
                                        qt[64 * h:64 * h + 64, hp, qs],
                                        start=True, stop=True,
                                        tile_position=(64 * h, 0))
                                ex = exp_pool.tile([128, 2, SC], BF16,
                                                   tag="ex", name="ex")
                                nc.scalar.activation(ex[:], sp[:], EXPF,
                                                     scale=1.0 / H)
                                for h in range(2):
                                    nc.tensor.matmul(
                                        pvs[h][:], vx[:, kb, 2 * hp + h, :],
                                        ex[:, h, :],
                                        start=(kb == 0), stop=(kb == NKB - 1),
                                        skip_group_check=True)
                                # hide next head-pair's Q/K projections in
                                # the ACT-paced slack of the kb loop
                                if hp < NHP - 1 and kb in (5, 11):
                                    w_sb, dst = ((wk_sb, kt) if kb == 5
                                                 else (wq_sb, qt))
                                    qk_group(hp + 1, w_sb, dst, qc)
                            for h in range(2):
                                dr = nrm.tile([1, SC], F32, tag="dr",
                                              name="dr")
                                nc.vector.tensor_copy(dr[:],
                                                      pvs[h][DH:DH + 1, :])
                                pvc = nrm.tile([DH, SC], F32, tag="pvc",
                                               name="pvc")
                                nc.vector.tensor_copy(pvc[:],
                                                      pvs[h][0:DH, :])
                                den = nrm.tile([1, SC], F32, tag="den",
                                               name="den")
                                nc.vector.reciprocal_approx_fast(den[:],
                                                                 dr[:])
                                bc = nrm.tile([DH, SC], F32, tag="bc",
                                              name="bc")
                                nc.gpsimd.partition_broadcast(bc[:], den[:])
                                nc.vector.tensor_mul(
                                    ot[64 * h:64 * h + 64, hp, qs],
                                    pvc[:], bc[:])
                        nc.gpsimd.dma_start(out_t[:, hp, :], ot[:, hp, :])

    nc.compile()
    return nc


def run(inputs, trace=False):
    x = np.asarray(inputs["encoder_input"], dtype=np.float32)
    Wq = np.asarray(inputs["Wq"], dtype=np.float32)
    Wk = np.asarray(inputs["Wk"], dtype=np.float32)
    Wv = np.asarray(inputs["Wv"], dtype=np.float32)
    bf = ml_dtypes.bfloat16

    nc = _build()
    in_maps = []
    for c in range(NCORES):
        b, g = c // 2, c % 2
        cols = slice(g * DLOC, (g + 1) * DLOC)
        xT = x[b].T                                       # [1024, 2048]
        x4 = xT.reshape(NDT, 128, NSC, SC).transpose(1, 2, 0, 3)
        in_maps.append({
            "x4": np.ascontiguousarray(x4.astype(bf)),
            "Wq": np.ascontiguousarray(
                Wq[:, cols].reshape(NDT, 128, DLOC).transpose(1, 0, 2)
                .astype(bf)),
            "Wk": np.ascontiguousarray(
                Wk[:, cols].reshape(NDT, 128, DLOC).transpose(1, 0, 2)
                .astype(bf)),
            "Wv": np.ascontiguousarray(
                Wv[:, cols].reshape(NDT, 128, DLOC).transpose(1, 0, 2)
                .astype(bf)),
        })
    res = run_bass_kernel_spmd(nc, in_maps, core_ids=list(range(NCORES)),
                               trace=trace)
    out = np.empty((B, S, D), dtype=np.float32)
    for c in range(NCORES):
        b, g = c // 2, c % 2
        out[b, :, g * DLOC:(g + 1) * DLOC] = res.results[c]["outT"].T
    return out, res


def kernel(**inputs):
    out, _ = run(inputs, trace=False)
    return out


# revision 6
# speedup vs baseline: 2.3294x; 1.0087x over previous
"""Multi-head self-attention (B=4, S=2048, D=1024, H=16) on 8 trn2 NeuronCores.

Sharding: core c -> batch b = c//2, head-group g = c%2 (8 heads, 512 of the
1024 output/QKV columns). Each core computes Q/K/V projections for its slice
and full attention for its 8 heads. Host does layout prep (bf16 conversion,
x transpose, W column slices) and the final gather/transpose - no collectives.

v2 vs v1: all matmuls in bf16 (v1's float32r lowered to fp32_mode=HIGH
multi-pass matmuls at ~2-4x the cost; the trace showed PE 100% busy for
680us of the 830us attn phase). Single x pass (V uses x as stationary, Q/K
as moving, same resident SBUF copy). Exp batched to 1024 free-dim per
ACTIVATE (halves the per-instruction overhead on the Scalar engine).

Per-core pipeline:
  phase 1 (qkv): V[s,dloc] psum groups (stationary=x chunk, moving=Wv) ->
           vx[128,16,8,65] bf16 with a ones column per head (PV denominator);
           KT/QT[128(2 heads x 64 dh), hp, s] bf16 via (stationary=W chunk,
           moving=x chunk) psum groups.
  phase 2 (attn): per (hp, qc): 16 k-blocks:
           scoresT pair (2 heads, tile_position row split) -> sp[128,2,512]
           psum (2 banks); one ACTIVATE Exp(scale=1/16) -> ex[128,2,512]
           bf16; 2 PV matmuls accumulate pv[65,512] (row 64 = denominator);
           then normalize: out = pv[0:64] * partition_broadcast(1/pv[64]).
"""
import ml_dtypes
import numpy as np

import concourse.bacc as bacc
import concourse.mybir as mybir
import concourse.tile as tile
from concourse.bass_utils import run_bass_kernel_spmd

B, S, D, H = 4, 2048, 1024, 16
DH = D // H            # 64
NCORES = 8
HLOC = H // 2          # 8 heads per core
DLOC = HLOC * DH       # 512 output cols per core
F32 = mybir.dt.float32
BF16 = mybir.dt.bfloat16
EXPF = mybir.ActivationFunctionType.Exp

SC = 512               # s-chunk in phase 1
NSC = S // SC          # 4
NKB = S // 128         # 16 k-blocks
NDT = D // 128         # 8 contraction tiles for QKV
NHP = HLOC // 2        # 4 head pairs


def _build():
    nc = bacc.Bacc("TRN2", target_bir_lowering=False, debug=False,
                   num_devices=NCORES)
    # x: [p, sc, o, s'] with d = o*128+p, s = sc*512+s' (contiguous DMA rows)
    x_h = nc.dram_tensor("x4", [128, NSC, NDT, SC], BF16,
                         kind="ExternalInput").ap()
    wq_h = nc.dram_tensor("Wq", [128, NDT, DLOC], BF16,
                          kind="ExternalInput").ap()
    wk_h = nc.dram_tensor("Wk", [128, NDT, DLOC], BF16,
                          kind="ExternalInput").ap()
    wv_h = nc.dram_tensor("Wv", [128, NDT, DLOC], BF16,
                          kind="ExternalInput").ap()
    out = nc.dram_tensor("outT", [DLOC, S], F32, kind="ExternalOutput").ap()
    out_t = out.rearrange("(o p) s -> p o s", p=128)      # [128, 4, 2048]

    with tile.TileContext(nc) as tc:
        with tc.tile_pool(name="persist", bufs=1) as keep:
            x_sb = keep.tile([128, NSC, NDT, SC], BF16)
            wq_sb = keep.tile([128, NDT, DLOC], BF16)
            wk_sb = keep.tile([128, NDT, DLOC], BF16)
            wv_sb = keep.tile([128, NDT, DLOC], BF16)
            vx = keep.tile([128, NKB, HLOC, DH + 1], BF16)
            kt = keep.tile([128, NHP, S], BF16)   # [2x64 dh, hp, s]
            qt = keep.tile([128, NHP, S], BF16)
            ot = keep.tile([128, NHP, S], F32)

            nc.sync.dma_start(wv_sb[:], wv_h)
            for sc in range(NSC):
                nc.sync.dma_start(x_sb[:, sc], x_h[:, sc])
            nc.sync.dma_start(wk_sb[:], wk_h)
            nc.sync.dma_start(wq_sb[:], wq_h)
            ones_t = keep.tile([128, NKB, HLOC], BF16)
            nc.vector.memset(ones_t[:], 1.0)
            nc.vector.tensor_copy(vx[:, :, :, DH], ones_t[:])

            with tc.tile_pool(name="p1ps", bufs=2, space="PSUM") as p1ps, \
                 tc.tile_pool(name="spp", bufs=2, space="PSUM") as spp, \
                 tc.tile_pool(name="pvp", bufs=1, space="PSUM") as pvp, \
                 tc.tile_pool(name="exp", bufs=4) as exp_pool, \
                 tc.tile_pool(name="nrm", bufs=2) as nrm:

                def qk_group(hp, w_sb, dst, sc):
                    cs = slice(hp * 128, (hp + 1) * 128)
                    ss = slice(sc * SC, (sc + 1) * SC)
                    ps = p1ps.tile([128, SC], F32, tag="p1", name="psqk")
                    for dt_i in range(NDT):
                        nc.tensor.matmul(
                            ps[:],
                            w_sb[:, dt_i, cs],
                            x_sb[:, sc, dt_i, :],
                            start=(dt_i == 0), stop=(dt_i == NDT - 1),
                        )
                    nc.vector.tensor_copy(dst[:, hp, ss], ps[:])

                # -------- phase 1 head start: V (all) + Q/K for hp0 ------
                with nc.named_scope("qkv"):
                    for sc in range(NSC):
                        for sb in range(SC // 128):
                            ps = p1ps.tile([128, DLOC], F32, tag="p1",
                                           name="psv")
                            for dt_i in range(NDT):
                                nc.tensor.matmul(
                                    ps[:],
                                    x_sb[:, sc, dt_i,
                                         sb * 128:(sb + 1) * 128],
                                    wv_sb[:, dt_i, :],
                                    start=(dt_i == 0), stop=(dt_i == NDT - 1),
                                )
                            s_idx = sc * (SC // 128) + sb
                            nc.vector.tensor_copy(
                                vx[:, s_idx, :, 0:DH],
                                ps[:].rearrange("p (h d) -> p h d", h=HLOC))
                    for sc in range(NSC):
                        qk_group(0, wk_sb, kt, sc)
                    qk_group(0, wq_sb, qt, 0)

                # -------- phase 2: attention (QK for hp+1 interleaved) ---
                with nc.named_scope("attn"):
                    for hp in range(NHP):
                        for qc in range(NSC):
                            qs = slice(qc * SC, (qc + 1) * SC)
                            pvs = [pvp.tile([DH + 1, SC], F32, tag=f"pv{h}",
                                            name=f"pv{h}") for h in range(2)]
                            for kb in range(NKB):
                                ks = slice(kb * 128, (kb + 1) * 128)
                                sp = spp.tile([128, 2, SC], F32, tag="sp",
                                              name="sp")
                                for h in range(2):
                                    nc.tensor.matmul(
                                        sp[:, h, :],
                                        kt[64 * h:64 * h + 64, hp, ks],
                                        qt[64 * h:64 * h + 64, hp, qs],
                                        start=True, stop=True,
                                        tile_position=(64 * h, 0))
                                ex = exp_pool.tile([128, 2, SC], BF16,
                                                   tag="ex", name="ex")
                                nc.scalar.activation(ex[:], sp[:], EXPF,
                                                     scale=1.0 / H)
                                for h in range(2):
                                    nc.tensor.matmul(
                                        pvs[h][:], vx[:, kb, 2 * hp + h, :],
                                        ex[:, h, :],
                                        start=(kb == 0), stop=(kb == NKB - 1),
                                        skip_group_check=True)
                                # hide remaining Q/K projections in the
                                # ACT-paced slack of the kb loop
                                if hp == 0 and kb == 3 and qc < NSC - 1:
                                    qk_group(0, wq_sb, qt, qc + 1)
                                if hp < NHP - 1 and kb == 8:
                                    qk_group(hp + 1, wk_sb, kt, qc)
                                if hp < NHP - 1 and kb == 13:
                                    qk_group(hp + 1, wq_sb, qt, qc)
                            for h in range(2):
                                dr = nrm.tile([1, SC], F32, tag="dr",
                                              name="dr")
                                nc.vector.tensor_copy(dr[:],
                                                      pvs[h][DH:DH + 1, :])
                                pvc = nrm.tile([DH, SC], F32, tag="pvc",
                                               name="pvc")
                                nc.vector.tensor_copy(pvc[:],
                                                      pvs[h][0:DH, :])
                                den = nrm.tile([1, SC], F32, tag="den",
                                               name="den")
                                nc.vector.reciprocal_approx_fast(den[:],
                                                                 dr[:])
                                bc = nrm.tile([DH, SC], F32, tag="bc",
                                              name="bc")
                                nc.gpsimd.partition_broadcast(bc[:], den[:])
                                nc.vector.tensor_mul(
                                    ot[64 * h:64 * h + 64, hp, qs],
                                    pvc[:], bc[:])
                            nc.sync.dma_start(out_t[:, hp, qs],
                                              ot[:, hp, qs])

    nc.compile()
    return nc


def run(inputs, trace=False):
    x = np.asarray(inputs["encoder_input"], dtype=np.float32)
    Wq = np.asarray(inputs["Wq"], dtype=np.float32)
    Wk = np.asarray(inputs["Wk"], dtype=np.float32)
    Wv = np.asarray(inputs["Wv"], dtype=np.float32)
    bf = ml_dtypes.bfloat16

    nc = _build()
    in_maps = []
    for c in range(NCORES):
        b, g = c // 2, c % 2
        cols = slice(g * DLOC, (g + 1) * DLOC)
        xT = x[b].T                                       # [1024, 2048]
        x4 = xT.reshape(NDT, 128, NSC, SC).transpose(1, 2, 0, 3)
        in_maps.append({
            "x4": np.ascontiguousarray(x4.astype(bf)),
            "Wq": np.ascontiguousarray(
                Wq[:, cols].reshape(NDT, 128, DLOC).transpose(1, 0, 2)
                .astype(bf)),
            "Wk": np.ascontiguousarray(
                Wk[:, cols].reshape(NDT, 128, DLOC).transpose(1, 0, 2)
                .astype(bf)),
            "Wv": np.ascontiguousarray(
                Wv[:, cols].reshape(NDT, 128, DLOC).transpose(1, 0, 2)
                .astype(bf)),
        })
    res = run_bass_kernel_spmd(nc, in_maps, core_ids=list(range(NCORES)),
                               trace=trace)
    out = np.empty((B, S, D), dtype=np.float32)
    for c in range(NCORES):
        b, g = c // 2, c % 2
        out[b, :, g * DLOC:(g + 1) * DLOC] = res.results[c]["outT"].T
    return out, res


def kernel(**inputs):
    out, _ = run(inputs, trace=False)
    return out


# revision 7
# speedup vs baseline: 2.3383x; 1.0038x over previous
"""Multi-head self-attention (B=4, S=2048, D=1024, H=16) on 8 trn2 NeuronCores.

Sharding: core c -> batch b = c//2, head-group g = c%2 (8 heads, 512 of the
1024 output/QKV columns). Each core computes Q/K/V projections for its slice
and full attention for its 8 heads. Host does layout prep (bf16 conversion,
x transpose, W column slices) and the final gather/transpose - no collectives.

v2 vs v1: all matmuls in bf16 (v1's float32r lowered to fp32_mode=HIGH
multi-pass matmuls at ~2-4x the cost; the trace showed PE 100% busy for
680us of the 830us attn phase). Single x pass (V uses x as stationary, Q/K
as moving, same resident SBUF copy). Exp batched to 1024 free-dim per
ACTIVATE (halves the per-instruction overhead on the Scalar engine).

Per-core pipeline:
  phase 1 (qkv): V[s,dloc] psum groups (stationary=x chunk, moving=Wv) ->
           vx[128,16,8,65] bf16 with a ones column per head (PV denominator);
           KT/QT[128(2 heads x 64 dh), hp, s] bf16 via (stationary=W chunk,
           moving=x chunk) psum groups.
  phase 2 (attn): per (hp, qc): 16 k-blocks:
           scoresT pair (2 heads, tile_position row split) -> sp[128,2,512]
           psum (2 banks); one ACTIVATE Exp(scale=1/16) -> ex[128,2,512]
           bf16; 2 PV matmuls accumulate pv[65,512] (row 64 = denominator);
           then normalize: out = pv[0:64] * partition_broadcast(1/pv[64]).
"""
import ml_dtypes
import numpy as np

import concourse.bacc as bacc
import concourse.mybir as mybir
import concourse.tile as tile
from concourse.bass_utils import run_bass_kernel_spmd

B, S, D, H = 4, 2048, 1024, 16
DH = D // H            # 64
NCORES = 8
HLOC = H // 2          # 8 heads per core
DLOC = HLOC * DH       # 512 output cols per core
F32 = mybir.dt.float32
BF16 = mybir.dt.bfloat16
EXPF = mybir.ActivationFunctionType.Exp

SC = 512               # s-chunk in phase 1
NSC = S // SC          # 4
NKB = S // 128         # 16 k-blocks
NDT = D // 128         # 8 contraction tiles for QKV
NHP = HLOC // 2        # 4 head pairs


def _build():
    nc = bacc.Bacc("TRN2", target_bir_lowering=False, debug=False,
                   num_devices=NCORES)
    # x: [p, sc, sb, o, j] with d = o*128+p, s = sc*512+sb*128+j
    x_h = nc.dram_tensor("x4", [128, NSC, 4, NDT, 128], BF16,
                         kind="ExternalInput").ap()
    wq_h = nc.dram_tensor("Wq", [128, NDT, DLOC], BF16,
                          kind="ExternalInput").ap()
    wk_h = nc.dram_tensor("Wk", [128, NDT, DLOC], BF16,
                          kind="ExternalInput").ap()
    wv_h = nc.dram_tensor("Wv", [128, NDT, DLOC], BF16,
                          kind="ExternalInput").ap()
    out = nc.dram_tensor("outT", [DLOC, S], F32, kind="ExternalOutput").ap()
    out_t = out.rearrange("(o p) s -> p o s", p=128)      # [128, 4, 2048]

    with tile.TileContext(nc) as tc:
        with tc.tile_pool(name="persist", bufs=1) as keep:
            x_sb = keep.tile([128, NSC, 4, NDT, 128], BF16)
            wq_sb = keep.tile([128, NDT, DLOC], BF16)
            wk_sb = keep.tile([128, NDT, DLOC], BF16)
            wv_sb = keep.tile([128, NDT, DLOC], BF16)
            vx = keep.tile([128, NKB, HLOC, DH + 1], BF16)
            kt = keep.tile([128, NHP, S], BF16)   # [2x64 dh, hp, s]
            qt = keep.tile([128, NHP, S], BF16)
            ot = keep.tile([128, NHP, S], F32)

            nc.sync.dma_start(wv_sb[:], wv_h)
            for sb in range(4):
                nc.sync.dma_start(x_sb[:, 0, sb], x_h[:, 0, sb])
            nc.sync.dma_start(wk_sb[:], wk_h)
            for sc in range(1, NSC):
                nc.sync.dma_start(x_sb[:, sc], x_h[:, sc])
            nc.sync.dma_start(wq_sb[:], wq_h)
            ones_t = keep.tile([128, NKB, HLOC], BF16)
            nc.vector.memset(ones_t[:], 1.0)
            nc.vector.tensor_copy(vx[:, :, :, DH], ones_t[:])

            with tc.tile_pool(name="p1ps", bufs=2, space="PSUM") as p1ps, \
                 tc.tile_pool(name="spp", bufs=2, space="PSUM") as spp, \
                 tc.tile_pool(name="pvp", bufs=1, space="PSUM") as pvp, \
                 tc.tile_pool(name="exp", bufs=4) as exp_pool, \
                 tc.tile_pool(name="nrm", bufs=2) as nrm:

                pending = {}

                def qk_group(hp, w_sb, dst, sc, half=None):
                    cs = slice(hp * 128, (hp + 1) * 128)
                    ss = slice(sc * SC, (sc + 1) * SC)
                    key = (hp, id(w_sb), sc)
                    if half == 1:
                        ps = pending.pop(key)
                        dts = range(NDT // 2, NDT)
                    else:
                        ps = p1ps.tile([128, SC], F32, tag="p1", name="psqk")
                        dts = range(NDT) if half is None else range(NDT // 2)
                        if half == 0:
                            pending[key] = ps
                    for dt_i in dts:
                        nc.tensor.matmul(
                            ps[:],
                            w_sb[:, dt_i, cs],
                            x_sb[:, sc, :, dt_i, :],
                            start=(dt_i == 0), stop=(dt_i == NDT - 1),
                            skip_group_check=True,
                        )
                    if half != 0:
                        nc.vector.tensor_copy(dst[:, hp, ss], ps[:])

                # -------- phase 1 head start: V (all) + Q/K for hp0 ------
                with nc.named_scope("qkv"):
                    for sc in range(NSC):
                        for sb in range(SC // 128):
                            ps = p1ps.tile([128, DLOC], F32, tag="p1",
                                           name="psv")
                            for dt_i in range(NDT):
                                nc.tensor.matmul(
                                    ps[:],
                                    x_sb[:, sc, sb, dt_i, :],
                                    wv_sb[:, dt_i, :],
                                    start=(dt_i == 0), stop=(dt_i == NDT - 1),
                                )
                            s_idx = sc * (SC // 128) + sb
                            nc.vector.tensor_copy(
                                vx[:, s_idx, :, 0:DH],
                                ps[:].rearrange("p (h d) -> p h d", h=HLOC))
                    for sc in range(NSC):
                        qk_group(0, wk_sb, kt, sc)
                    qk_group(0, wq_sb, qt, 0)

                # -------- phase 2: attention (QK for hp+1 interleaved) ---
                with nc.named_scope("attn"):
                    for hp in range(NHP):
                        for qc in range(NSC):
                            qs = slice(qc * SC, (qc + 1) * SC)
                            pvs = [pvp.tile([DH + 1, SC], F32, tag=f"pv{h}",
                                            name=f"pv{h}") for h in range(2)]
                            for kb in range(NKB):
                                ks = slice(kb * 128, (kb + 1) * 128)
                                sp = spp.tile([128, 2, SC], F32, tag="sp",
                                              name="sp")
                                for h in range(2):
                                    nc.tensor.matmul(
                                        sp[:, h, :],
                                        kt[64 * h:64 * h + 64, hp, ks],
                                        qt[64 * h:64 * h + 64, hp, qs],
                                        start=True, stop=True,
                                        tile_position=(64 * h, 0))
                                ex = exp_pool.tile([128, 2, SC], BF16,
                                                   tag="ex", name="ex")
                                nc.scalar.activation(ex[:], sp[:], EXPF,
                                                     scale=1.0 / H)
                                for h in range(2):
                                    nc.tensor.matmul(
                                        pvs[h][:], vx[:, kb, 2 * hp + h, :],
                                        ex[:, h, :],
                                        start=(kb == 0), stop=(kb == NKB - 1),
                                        skip_group_check=True)
                                # hide remaining Q/K projections in the
                                # ACT-paced slack of the kb loop, 4 matmuls
                                # at a time so scores never fall behind
                                if hp == 0 and qc < NSC - 1:
                                    if kb == 2:
                                        qk_group(0, wq_sb, qt, qc + 1, half=0)
                                    elif kb == 4:
                                        qk_group(0, wq_sb, qt, qc + 1, half=1)
                                if hp < NHP - 1:
                                    if kb == 7:
                                        qk_group(hp + 1, wk_sb, kt, qc, half=0)
                                    elif kb == 9:
                                        qk_group(hp + 1, wk_sb, kt, qc, half=1)
                                    elif kb == 12:
                                        qk_group(hp + 1, wq_sb, qt, qc, half=0)
                                    elif kb == 14:
                                        qk_group(hp + 1, wq_sb, qt, qc, half=1)
                            for h in range(2):
                                dr = nrm.tile([1, SC], F32, tag="dr",
                                              name="dr")
                                nc.vector.tensor_copy(dr[:],
                                                      pvs[h][DH:DH + 1, :])
                                pvc = nrm.tile([DH, SC], F32, tag="pvc",
                                               name="pvc")
                                nc.vector.tensor_copy(pvc[:],
                                                      pvs[h][0:DH, :])
                                den = nrm.tile([1, SC], F32, tag="den",
                                               name="den")
                                nc.vector.reciprocal_approx_fast(den[:],
                                                                 dr[:])
                                bc = nrm.tile([DH, SC], F32, tag="bc",
                                              name="bc")
                                nc.gpsimd.partition_broadcast(bc[:], den[:])
                                nc.vector.tensor_mul(
                                    ot[64 * h:64 * h + 64, hp, qs],
                                    pvc[:], bc[:])
                            nc.sync.dma_start(out_t[:, hp, qs],
                                              ot[:, hp, qs])

    nc.compile()
    return nc


def run(inputs, trace=False):
    x = np.asarray(inputs["encoder_input"], dtype=np.float32)
    Wq = np.asarray(inputs["Wq"], dtype=np.float32)
    Wk = np.asarray(inputs["Wk"], dtype=np.float32)
    Wv = np.asarray(inputs["Wv"], dtype=np.float32)
    bf = ml_dtypes.bfloat16

    nc = _build()
    in_maps = []
    for c in range(NCORES):
        b, g = c // 2, c % 2
        cols = slice(g * DLOC, (g + 1) * DLOC)
        xT = x[b].T                                       # [1024, 2048]
        x4 = (xT.reshape(NDT, 128, NSC, 4, 128)
              .transpose(1, 2, 3, 0, 4))
        in_maps.append({
            "x4": np.ascontiguousarray(x4.astype(bf)),
            "Wq": np.ascontiguousarray(
                Wq[:, cols].reshape(NDT, 128, DLOC).transpose(1, 0, 2)
                .astype(bf)),
            "Wk": np.ascontiguousarray(
                Wk[:, cols].reshape(NDT, 128, DLOC).transpose(1, 0, 2)
                .astype(bf)),
            "Wv": np.ascontiguousarray(
                Wv[:, cols].reshape(NDT, 128, DLOC).transpose(1, 0, 2)
                .astype(bf)),
        })
    res = run_bass_kernel_spmd(nc, in_maps, core_ids=list(range(NCORES)),
                               trace=trace)
    out = np.empty((B, S, D), dtype=np.float32)
    for c in range(NCORES):
        b, g = c // 2, c % 2
        out[b, :, g * DLOC:(g + 1) * DLOC] = res.results[c]["outT"].T
    return out, res


def kernel(**inputs):
    out, _ = run(inputs, trace=False)
    return out
